# revision 33
# baseline (speedup 1.0000x reference)
"""Trainium2 Bass kernel for an 8-level circular DWT (forward + inverse).

The reference computes an 8-level periodized DWT (8-tap filters derived from
`scaling`) and returns (denoised, concat(coeffs)).  The inverse transform is
applied with no thresholding, so for orthonormal QMF filters (the DB4 bank
the reference ships) reconstruction is exactly the identity: denoised == x.
The kernel verifies that condition numerically and short-circuits the inverse
to a host-side copy; the forward transform runs on 8 NeuronCores,
data-parallel over rows (64 rows/core).

Device math: levels 0-3 are fused into ONE banded-matmul pass over x
("stage 1") using composite filters (up to 106 taps, stride 16 for the
level-3 outputs).  Output block c (128 outputs: 64 d0 | 32 d1 | 16 d2 |
8 d3 | 8 a3, at fixed partition segments) is

    psum[:, c] = M.T @ X_c + C.T @ X_{c-1}          (X_b = x[128b .. 128b+128))

so each input block is streamed exactly twice and both stationaries are
block-independent.  Matmuls are batched M,M,M,M / C,C,C,C so LDWEIGHTS
amortizes.  PSUM is evacuated f32->f16 by full-128-partition copies into S
with pi-major column order (block 16u + pi stored at col 32*pi + u), so the
a3 stripe (partitions 120:128) forms 64B-contiguous runs per pi.  One
SBUF->SBUF DMA per 16-row quarter then remaps the stripes into
X4[16q + pi, r, u] = a3[128u + 8pi + q] and stage 2 applies the identical
fused scheme for levels 4-7 on X4 (M2/C2 rows permuted to match the (q,pi)
partition order; 3 matmuls per quarter: M2, C2-wrap, C2).  Matmuls run in
float16 (full PE rate); PSUM accumulates fp32.
"""

import sys
from contextlib import ExitStack

for _p in ("/opt/trn_rl_repo", "/root/.axon_site/_ro/trn_rl_repo"):
    if _p not in sys.path:
        sys.path.append(_p)

import numpy as np

import concourse.bacc as bacc
import concourse.mybir as mybir
import concourse.tile as tile
from concourse.bass_utils import run_bass_kernel_spmd

F32 = mybir.dt.float32
F16 = mybir.dt.float16

N_ROWS = 512          # total rows
N0 = 65536            # row length (power of two: reference pad is a no-op)
LEVELS = 8
N_CORES = 8
ROWS = N_ROWS // N_CORES   # rows per core
NB = N0 // 128             # 128-blocks per row (512)
NB2 = NB // 16             # stage-2 blocks per row (32)
CHUNKS = (4, 4, 4, 4, 8, 8, 8, 8, 4, 4, 4, 4)  # stage-1 rows per chunk
# chunk idx -> stage-2 piece (row0, nrows); emitted one chunk after the
# piece's rows are evacuated so its matmuls never stall the PE FIFO
QBOUND = {4: (0, 16), 6: (16, 16), 8: (32, 16), 10: (48, 8), 11: (56, 8)}

# output partition segments within a 128-output block (stage 1 and stage 2)
SEG = ((0, 64, 2), (64, 32, 4), (96, 16, 8), (112, 8, 16), (120, 8, 16))


# ----------------------------- host-side math -----------------------------

def _wavelet(s):
    g = s[::-1].copy()
    sign = np.where(np.arange(s.shape[-1]) % 2 == 1, -1.0, 1.0).astype(g.dtype)
    return g * sign


def _make_mc(filters):
    """Fused 4-level stationaries [M, C] (128x128 f64 each, [p_in, m] lhsT).

    filters: (4, 8) scaling rows for the 4 levels of this stage.  Output
    block families: d0 (stride 2, 64/block), d1 (4, 32), d2 (8, 16),
    d3 (16, 8), a3 (16, 8) at partition bases 0/64/96/112/120.
    out[j] = sum_t g[t] x[s*j - t]; in-block index i = s*q - t for block
    slot q; i < 0 reads the previous block via C.
    """
    fs = [np.asarray(f, dtype=np.float64) for f in filters]
    ws = [_wavelet(f) for f in fs]
    # P[l] = taps of a_l w.r.t. stage input: P_l[2^l m + t] += s_l[m] P_{l-1}[t]
    P = [np.array([1.0])]
    for lvl in range(3):
        q = np.zeros((1 << lvl) * 7 + len(P[-1]), dtype=np.float64)
        for m in range(8):
            q[(1 << lvl) * m:(1 << lvl) * m + len(P[-1])] += fs[lvl][m] * P[-1]
        P.append(q)
    taps = []
    for lvl in range(4):  # d-taps per level
        g = np.zeros((1 << lvl) * 7 + len(P[lvl]), dtype=np.float64)
        for m in range(8):
            g[(1 << lvl) * m:(1 << lvl) * m + len(P[lvl])] += ws[lvl][m] * P[lvl]
        taps.append(g)
    ga = np.zeros(8 * 7 + len(P[3]), dtype=np.float64)  # a3-taps
    for m in range(8):
        ga[8 * m:8 * m + len(P[3])] += fs[3][m] * P[3]
    taps.append(ga)

    M = np.zeros((128, 128), dtype=np.float64)
    C = np.zeros((128, 128), dtype=np.float64)
    for (pbase, cnt, s), g in zip(SEG, taps):
        for q in range(cnt):
            for t in range(len(g)):
                i = s * q - t
                if i >= 0:
                    M[i, pbase + q] += g[t]
                else:
                    C[i + 128, pbase + q] += g[t]
    return M, C


def _make_wmat(scaling):
    s = np.asarray(scaling, dtype=np.float64)
    M1, C1 = _make_mc(s[0:4])
    M2, C2 = _make_mc(s[4:8])
    # rebuild permutation matmuls: X4[8pi+q] <- S partition 120+q, pi-group
    WP = np.zeros((128, 16 * 128))
    for pi in range(16):
        for q in range(8):
            WP[120 + q, 128 * pi + 8 * pi + q] = 1.0
    return np.concatenate([M1, C1, M2, C2, WP], axis=1).astype(np.float16)


def _pack_x_shard(x_rows):
    rows, n = x_rows.shape
    nb = n // 128
    blocks = x_rows.astype(np.float16).reshape(rows, nb, 128).transpose(2, 0, 1)
    xt = np.empty((128, rows, nb + 1), dtype=np.float16)
    xt[:, :, 1:] = blocks
    xt[:, :, 0] = blocks[:, :, nb - 1]           # circular halo column
    return np.ascontiguousarray(xt.reshape(128, rows * (nb + 1)))


def _unpack_stage(arr, rows, nblk, pi_major):
    """[P, rows*nblk-cols] device layout -> [rows, nblk, P] block-major."""
    p = arr.shape[0]
    if pi_major:  # device cols (pi, u); block b = 16u + pi
        a = arr.reshape(p, rows, 16, nblk // 16).transpose(1, 3, 2, 0)
    else:
        a = arr.reshape(p, rows, nblk).transpose(1, 2, 0)
    return np.ascontiguousarray(a.reshape(rows, nblk, p))


def _is_orthonormal_qmf(scaling):
    s = np.asarray(scaling, dtype=np.float64)
    if s.shape != (LEVELS, 8):
        return False
    for lvl in range(LEVELS):
        f = s[lvl]
        for m in range(4):
            v = np.dot(f[: 8 - 2 * m], f[2 * m:])
            if abs(v - (1.0 if m == 0 else 0.0)) > 1e-4:
                return False
    return True


def _dwt_backward_numpy(ds, a, scaling):
    """Fallback inverse transform (float64 FFT) for non-orthonormal filters."""
    a = np.asarray(a, dtype=np.float64)
    for lvl in reversed(range(LEVELS)):
        s = np.asarray(scaling[lvl], dtype=np.float64)
        w = _wavelet(s)
        d = np.asarray(ds[lvl], dtype=np.float64)
        n = d.shape[-1] * 2
        fd = np.zeros((d.shape[0], n))
        fd[:, ::2] = d
        fa = np.zeros((a.shape[0], n))
        fa[:, ::2] = a
        a = (np.fft.irfft(np.fft.rfft(fd, axis=-1)
                          * np.conj(np.fft.rfft(w, n=n)), n=n, axis=-1)
             + np.fft.irfft(np.fft.rfft(fa, axis=-1)
                            * np.conj(np.fft.rfft(s, n=n)), n=n, axis=-1))
    return a


# ----------------------------- device kernel ------------------------------

def _build_dwt(tc, xt, xhead, wmat, st1, st2, rows=ROWS):
    nc = tc.nc
    xt3 = xt.rearrange("p (r b) -> p r b", b=NB + 1)
    st1r = st1.rearrange("p (r c) -> p r c", c=NB)
    st2r = st2.rearrange("p (r c) -> p r c", c=NB2)

    with ExitStack() as ctx:
        wpool = ctx.enter_context(tc.tile_pool(name="wpool", bufs=1))
        xpool = ctx.enter_context(tc.tile_pool(name="xpool",
                                               bufs=len(CHUNKS)))
        spool = ctx.enter_context(tc.tile_pool(name="spool", bufs=1))
        opool = ctx.enter_context(tc.tile_pool(name="opool", bufs=2))
        p1pool = ctx.enter_context(tc.tile_pool(name="p1pool", bufs=6,
                                                space="PSUM"))
        p2pool = ctx.enter_context(tc.tile_pool(name="p2pool", bufs=2,
                                                space="PSUM"))

        # weights on the (otherwise idle at start) scalar queue
        W = wpool.tile([128, 512], F16, name="Wsb")
        nc.scalar.dma_start(W[:], wmat[:, 0:512])
        M1, C1 = W[:, 0:128], W[:, 128:256]
        M2, C2 = W[:, 256:384], W[:, 384:512]
        WP = wpool.tile([128, 2048], F16, name="WPsb")
        nc.scalar.dma_start(WP[:], wmat[:, 512:2560])

        # tiny head load: lets the first psum tile start ~3us earlier
        xh = wpool.tile([128, 4, 129], F16, name="xh")
        nc.sync.dma_start(xh[:], xhead[:, :].rearrange("p (r b) -> p r b",
                                                       b=129))

        # all input loads issued up front, alternating queues so the early
        # chunks stream in parallel instead of serializing their latency
        xtiles = []
        r0 = 0
        for ci, nr in enumerate(CHUNKS):
            x0 = xpool.tile([128, nr, NB + 1], F16, tag="x0",
                            name=f"x{ci}")
            q = nc.sync if ci % 2 == 0 else nc.scalar
            q.dma_start(x0[:], xt3[:, r0:r0 + nr, :])
            xtiles.append(x0)
            r0 += nr

        # PE warm-up: keep HAM unthrottled until the first input lands.
        warm = wpool.tile([128, 256], F16, name="warm")
        nc.gpsimd.memset(warm[:], 0)
        pw = p1pool.tile([128, 256], F32, tag="ps1", name="pw")
        for _ in range(12):
            nc.tensor.matmul(pw[:], warm[:, 0:128], warm[:], start=True,
                             stop=True)

        # stage-1 output: S[p, r, pi, u] holds block 16u + pi of row r
        S = spool.tile([128, rows, 16, NB // 16], F16, name="S")
        # stage-2 input: X4[8pi + q, r, u] = a3[128u + 8pi + q]
        X4 = spool.tile([128, rows, NB2], F16, name="X4")

        def stage2(h, row0, nrows):
            """Levels 4-7 for rows [row0, row0+nrows) (PE rebuild + matmuls)."""
            rs = slice(row0, row0 + nrows)
            px = p1pool.tile([128, nrows, NB2], F32, tag="ps1", name="px4")
            for pi in range(16):
                nc.tensor.matmul(px[:], WP[:, 128 * pi:128 * pi + 128],
                                 S[:, rs, pi, :], start=(pi == 0),
                                 stop=(pi == 15))
            if h % 2 == 0:
                nc.scalar.copy(X4[:, rs, :], px[:])
            else:
                nc.vector.tensor_copy(X4[:, rs, :], px[:])
            ps = p2pool.tile([128, nrows, NB2], F32, tag="ps2", name="ps2")
            nc.tensor.matmul(ps[:], M2, X4[:, rs, :], start=True, stop=False)
            nc.tensor.matmul(ps[:, :, 0:1], C2, X4[:, rs, NB2 - 1:NB2],
                             start=False, stop=False)
            nc.tensor.matmul(ps[:, :, 1:NB2], C2, X4[:, rs, 0:NB2 - 1],
                             start=False, stop=True)
            o = opool.tile([128, nrows, NB2], F16, tag="st2", name="o2")
            if h % 2 == 0:
                nc.vector.tensor_copy(o[:], ps[:])
            else:
                nc.scalar.copy(o[:], ps[:])
            nc.scalar.dma_start(st2r[:, rs, :], o[:])

        r0 = 0
        for ci, nr in enumerate(CHUNKS):
            rs = slice(r0, r0 + nr)
            x0 = xtiles[ci]
            tb = 512 // nr              # blocks per psum tile
            spt = NB // tb              # psum tiles this chunk (= nr)
            packs = [range(t0, min(t0 + 4, spt)) for t0 in range(0, spt, 4)]
            if ci == 0:
                # finish psum tile 0 from the head load alone, then keep the
                # PE busy (HAM warm) until the first full chunk lands
                packs = [range(0, 1), range(1, spt)]
            for pk, tts in enumerate(packs):
                if ci == 0 and pk == 1:
                    for _ in range(14):
                        nc.tensor.matmul(pw[:], warm[:, 0:128], warm[:],
                                         start=True, stop=True)
                pss = {}
                for t in tts:           # M-batch (one LDWEIGHTS)
                    b0 = t * tb
                    xs = xh if ci == 0 and t == 0 and tb == 128 else x0
                    ps = pss[t] = p1pool.tile([128, nr, tb], F32, tag="ps1",
                                              name="ps1")
                    nc.tensor.matmul(ps[:], M1, xs[:, :, 1 + b0:1 + b0 + tb],
                                     start=True, stop=False)
                for t in tts:           # C-batch
                    b0 = t * tb
                    xs = xh if ci == 0 and t == 0 and tb == 128 else x0
                    nc.tensor.matmul(pss[t][:], C1, xs[:, :, b0:b0 + tb],
                                     start=False, stop=True)
                for t in tts:           # evac f32->f16, pi-major S cols
                    u0 = (t * tb) // 16
                    dst = S[:, rs, :, u0:u0 + tb // 16]
                    src = pss[t][:].rearrange("p r (u pi) -> p r pi u", pi=16)
                    if t % 2 == 0:
                        nc.vector.tensor_copy(dst, src)
                    else:
                        nc.scalar.copy(dst, src)
            nc.sync.dma_start(st1r[:, rs, :], S[0:120, rs, :, :])
            if ci in QBOUND:
                qr0, qnr = QBOUND[ci]
                stage2(qr0 // 16, qr0, qnr)
            r0 += nr


_MODULE_CACHE = {}


def _get_module():
    if "nc" in _MODULE_CACHE:
        return _MODULE_CACHE["nc"]
    nc = bacc.Bacc("TRN2", target_bir_lowering=False, debug=False,
                   num_devices=N_CORES)
    xt = nc.dram_tensor("xt", [128, ROWS * (NB + 1)], F16,
                        kind="ExternalInput").ap()
    xhead = nc.dram_tensor("xhead", [128, 4 * 129], F16,
                           kind="ExternalInput").ap()
    wmat = nc.dram_tensor("wmat", [128, 2560], F16, kind="ExternalInput").ap()
    st1 = nc.dram_tensor("st1", [120, ROWS * NB], F16,
                         kind="ExternalOutput").ap()
    st2 = nc.dram_tensor("st2", [128, ROWS * NB2], F16,
                         kind="ExternalOutput").ap()
    with tile.TileContext(nc) as tc:
        _build_dwt(tc, xt, xhead, wmat, st1, st2)
    nc.compile()
    _MODULE_CACHE["nc"] = nc
    return nc


def run(x, scaling, **spmd_kwargs):
    """Full pipeline.  Returns (denoised, coeffs, BassKernelResults)."""
    x = np.ascontiguousarray(np.asarray(x, dtype=np.float32))
    scaling = np.asarray(scaling, dtype=np.float32)
    assert x.shape == (N_ROWS, N0), x.shape
    assert scaling.shape == (LEVELS, 8), scaling.shape

    nc = _get_module()
    wmat = _make_wmat(scaling)
    in_maps = []
    for c in range(N_CORES):
        xp = _pack_x_shard(x[c * ROWS:(c + 1) * ROWS])
        xhead = np.ascontiguousarray(
            xp.reshape(128, ROWS, NB + 1)[:, 0:4, 0:129].reshape(128, -1))
        in_maps.append({"xt": xp, "xhead": xhead, "wmat": wmat})

    res = run_bass_kernel_spmd(nc, in_maps, core_ids=list(range(N_CORES)),
                               **spmd_kwargs)

    coeffs = np.empty((N_ROWS, N0), dtype=np.float32)
    ds_full = []
    offs = np.cumsum([0] + [N0 >> (l + 1) for l in range(LEVELS)])
    for c in range(N_CORES):
        rsl = slice(c * ROWS, (c + 1) * ROWS)
        b1 = _unpack_stage(res.results[c]["st1"], ROWS, NB, True)
        b2 = _unpack_stage(res.results[c]["st2"], ROWS, NB2, False)
        for lvl in range(LEVELS):
            blk, (p0, cnt, _) = (b1, SEG[lvl]) if lvl < 4 else \
                (b2, SEG[lvl - 4])
            coeffs[rsl, offs[lvl]:offs[lvl + 1]] = (
                blk[:, :, p0:p0 + cnt].reshape(ROWS, -1))
        coeffs[rsl, offs[8]:] = b2[:, :, 120:128].reshape(ROWS, -1)

    for lvl in range(LEVELS):
        ds_full.append(coeffs[:, offs[lvl]:offs[lvl + 1]])
    a_full = coeffs[:, offs[8]:]

    if _is_orthonormal_qmf(scaling):
        # Orthonormal QMF bank + untouched coefficients => the inverse
        # transform is exactly the identity (reference pad is a no-op).
        denoised = x.copy()
    else:
        denoised = _dwt_backward_numpy(ds_full, a_full, scaling).astype(np.float32)

    return denoised, coeffs, res


def kernel(x, scaling):
    denoised, coeffs, _ = run(x, scaling)
    return denoised, coeffs


# revision 35
# speedup vs baseline: 1.0067x; 1.0067x over previous
"""Trainium2 Bass kernel for an 8-level circular DWT (forward + inverse).

The reference computes an 8-level periodized DWT (8-tap filters derived from
`scaling`) and returns (denoised, concat(coeffs)).  The inverse transform is
applied with no thresholding, so for orthonormal QMF filters (the DB4 bank
the reference ships) reconstruction is exactly the identity: denoised == x.
The kernel verifies that condition numerically and short-circuits the inverse
to a host-side copy; the forward transform runs on 8 NeuronCores,
data-parallel over rows (64 rows/core).

Device math: levels 0-3 are fused into ONE banded-matmul pass over x
("stage 1") using composite filters (up to 106 taps, stride 16 for the
level-3 outputs).  Output block c (128 outputs: 64 d0 | 32 d1 | 16 d2 |
8 d3 | 8 a3, at fixed partition segments) is

    psum[:, c] = M.T @ X_c + C.T @ X_{c-1}          (X_b = x[128b .. 128b+128))

so each input block is streamed exactly twice and both stationaries are
block-independent.  Matmuls are batched M,M,M,M / C,C,C,C so LDWEIGHTS
amortizes.  PSUM is evacuated f32->f16 by full-128-partition copies into S
with pi-major column order (block 16u + pi stored at col 32*pi + u), so the
a3 stripe (partitions 120:128) forms 64B-contiguous runs per pi.  One
SBUF->SBUF DMA per 16-row quarter then remaps the stripes into
X4[16q + pi, r, u] = a3[128u + 8pi + q] and stage 2 applies the identical
fused scheme for levels 4-7 on X4 (M2/C2 rows permuted to match the (q,pi)
partition order; 3 matmuls per quarter: M2, C2-wrap, C2).  Matmuls run in
float16 (full PE rate); PSUM accumulates fp32.
"""

import sys
from contextlib import ExitStack

for _p in ("/opt/trn_rl_repo", "/root/.axon_site/_ro/trn_rl_repo"):
    if _p not in sys.path:
        sys.path.append(_p)

import numpy as np

import concourse.bacc as bacc
import concourse.mybir as mybir
import concourse.tile as tile
from concourse.bass_utils import run_bass_kernel_spmd

F32 = mybir.dt.float32
F16 = mybir.dt.float16

N_ROWS = 512          # total rows
N0 = 65536            # row length (power of two: reference pad is a no-op)
LEVELS = 8
N_CORES = 8
ROWS = N_ROWS // N_CORES   # rows per core
NB = N0 // 128             # 128-blocks per row (512)
NB2 = NB // 16             # stage-2 blocks per row (32)
CHUNKS = (4, 4, 4, 4, 8, 8, 8, 8, 4, 4, 4, 4)  # stage-1 rows per chunk
# chunk idx -> stage-2 piece (row0, nrows); emitted one chunk after the
# piece's rows are evacuated so its matmuls never stall the PE FIFO
QBOUND = {4: (0, 16), 6: (16, 16), 8: (32, 16), 10: (48, 8), 11: (56, 8)}

# output partition segments within a 128-output block (stage 1 and stage 2)
SEG = ((0, 64, 2), (64, 32, 4), (96, 16, 8), (112, 8, 16), (120, 8, 16))


# ----------------------------- host-side math -----------------------------

def _wavelet(s):
    g = s[::-1].copy()
    sign = np.where(np.arange(s.shape[-1]) % 2 == 1, -1.0, 1.0).astype(g.dtype)
    return g * sign


def _make_mc(filters):
    """Fused 4-level stationaries [M, C] (128x128 f64 each, [p_in, m] lhsT).

    filters: (4, 8) scaling rows for the 4 levels of this stage.  Output
    block families: d0 (stride 2, 64/block), d1 (4, 32), d2 (8, 16),
    d3 (16, 8), a3 (16, 8) at partition bases 0/64/96/112/120.
    out[j] = sum_t g[t] x[s*j - t]; in-block index i = s*q - t for block
    slot q; i < 0 reads the previous block via C.
    """
    fs = [np.asarray(f, dtype=np.float64) for f in filters]
    ws = [_wavelet(f) for f in fs]
    # P[l] = taps of a_l w.r.t. stage input: P_l[2^l m + t] += s_l[m] P_{l-1}[t]
    P = [np.array([1.0])]
    for lvl in range(3):
        q = np.zeros((1 << lvl) * 7 + len(P[-1]), dtype=np.float64)
        for m in range(8):
            q[(1 << lvl) * m:(1 << lvl) * m + len(P[-1])] += fs[lvl][m] * P[-1]
        P.append(q)
    taps = []
    for lvl in range(4):  # d-taps per level
        g = np.zeros((1 << lvl) * 7 + len(P[lvl]), dtype=np.float64)
        for m in range(8):
            g[(1 << lvl) * m:(1 << lvl) * m + len(P[lvl])] += ws[lvl][m] * P[lvl]
        taps.append(g)
    ga = np.zeros(8 * 7 + len(P[3]), dtype=np.float64)  # a3-taps
    for m in range(8):
        ga[8 * m:8 * m + len(P[3])] += fs[3][m] * P[3]
    taps.append(ga)

    M = np.zeros((128, 128), dtype=np.float64)
    C = np.zeros((128, 128), dtype=np.float64)
    for (pbase, cnt, s), g in zip(SEG, taps):
        for q in range(cnt):
            for t in range(len(g)):
                i = s * q - t
                if i >= 0:
                    M[i, pbase + q] += g[t]
                else:
                    C[i + 128, pbase + q] += g[t]
    return M, C


def _make_wmat(scaling):
    s = np.asarray(scaling, dtype=np.float64)
    M1, C1 = _make_mc(s[0:4])
    M2, C2 = _make_mc(s[4:8])
    # rebuild permutation matmuls: X4[8pi+q] <- S partition 120+q, pi-group
    WP = np.zeros((128, 16 * 128))
    for pi in range(16):
        for q in range(8):
            WP[120 + q, 128 * pi + 8 * pi + q] = 1.0
    return np.concatenate([M1, C1, M2, C2, WP], axis=1).astype(np.float16)


def _pack_x_shard(x_rows):
    rows, n = x_rows.shape
    nb = n // 128
    blocks = x_rows.astype(np.float16).reshape(rows, nb, 128).transpose(2, 0, 1)
    xt = np.empty((128, rows, nb + 1), dtype=np.float16)
    xt[:, :, 1:] = blocks
    xt[:, :, 0] = blocks[:, :, nb - 1]           # circular halo column
    return np.ascontiguousarray(xt.reshape(128, rows * (nb + 1)))


def _unpack_stage(arr, rows, nblk, pi_major):
    """[P, rows*nblk-cols] device layout -> [rows, nblk, P] block-major."""
    p = arr.shape[0]
    if pi_major:  # device cols (pi, u); block b = 16u + pi
        a = arr.reshape(p, rows, 16, nblk // 16).transpose(1, 3, 2, 0)
    else:
        a = arr.reshape(p, rows, nblk).transpose(1, 2, 0)
    return np.ascontiguousarray(a.reshape(rows, nblk, p))


def _is_orthonormal_qmf(scaling):
    s = np.asarray(scaling, dtype=np.float64)
    if s.shape != (LEVELS, 8):
        return False
    for lvl in range(LEVELS):
        f = s[lvl]
        for m in range(4):
            v = np.dot(f[: 8 - 2 * m], f[2 * m:])
            if abs(v - (1.0 if m == 0 else 0.0)) > 1e-4:
                return False
    return True


def _dwt_backward_numpy(ds, a, scaling):
    """Fallback inverse transform (float64 FFT) for non-orthonormal filters."""
    a = np.asarray(a, dtype=np.float64)
    for lvl in reversed(range(LEVELS)):
        s = np.asarray(scaling[lvl], dtype=np.float64)
        w = _wavelet(s)
        d = np.asarray(ds[lvl], dtype=np.float64)
        n = d.shape[-1] * 2
        fd = np.zeros((d.shape[0], n))
        fd[:, ::2] = d
        fa = np.zeros((a.shape[0], n))
        fa[:, ::2] = a
        a = (np.fft.irfft(np.fft.rfft(fd, axis=-1)
                          * np.conj(np.fft.rfft(w, n=n)), n=n, axis=-1)
             + np.fft.irfft(np.fft.rfft(fa, axis=-1)
                            * np.conj(np.fft.rfft(s, n=n)), n=n, axis=-1))
    return a


# ----------------------------- device kernel ------------------------------

def _build_dwt(tc, xt, xhead, wmat, st1, st2, rows=ROWS):
    nc = tc.nc
    xt3 = xt.rearrange("p (r b) -> p r b", b=NB + 1)
    st1r = st1.rearrange("p (r c) -> p r c", c=NB)
    st2r = st2.rearrange("p (r c) -> p r c", c=NB2)

    with ExitStack() as ctx:
        wpool = ctx.enter_context(tc.tile_pool(name="wpool", bufs=1))
        xpool = ctx.enter_context(tc.tile_pool(name="xpool",
                                               bufs=len(CHUNKS)))
        spool = ctx.enter_context(tc.tile_pool(name="spool", bufs=1))
        opool = ctx.enter_context(tc.tile_pool(name="opool", bufs=2))
        p1pool = ctx.enter_context(tc.tile_pool(name="p1pool", bufs=6,
                                                space="PSUM"))
        p2pool = ctx.enter_context(tc.tile_pool(name="p2pool", bufs=2,
                                                space="PSUM"))

        # weights on the (otherwise idle at start) scalar queue
        W = wpool.tile([128, 512], F16, name="Wsb")
        nc.scalar.dma_start(W[:], wmat[:, 0:512])
        M1, C1 = W[:, 0:128], W[:, 128:256]
        M2, C2 = W[:, 256:384], W[:, 384:512]
        WP = wpool.tile([128, 2048], F16, name="WPsb")
        nc.scalar.dma_start(WP[:], wmat[:, 512:2560])

        # tiny head load: lets the first psum tile start ~3us earlier
        xh = wpool.tile([128, 4, 129], F16, name="xh")
        nc.sync.dma_start(xh[:], xhead[:, :].rearrange("p (r b) -> p r b",
                                                       b=129))

        # all input loads issued up front, alternating queues so the early
        # chunks stream in parallel instead of serializing their latency
        xtiles = []
        r0 = 0
        for ci, nr in enumerate(CHUNKS):
            x0 = xpool.tile([128, nr, NB + 1], F16, tag="x0",
                            name=f"x{ci}")
            q = nc.gpsimd if ci in (1, 3) else nc.sync
            q.dma_start(x0[:], xt3[:, r0:r0 + nr, :])
            xtiles.append(x0)
            r0 += nr

        # PE warm-up: keep HAM unthrottled until the first input lands.
        warm = wpool.tile([128, 256], F16, name="warm")
        nc.gpsimd.memset(warm[:], 0)
        pw = p1pool.tile([128, 256], F32, tag="ps1", name="pw")
        for _ in range(12):
            nc.tensor.matmul(pw[:], warm[:, 0:128], warm[:], start=True,
                             stop=True)

        # stage-1 output: S[p, r, pi, u] holds block 16u + pi of row r
        S = spool.tile([128, rows, 16, NB // 16], F16, name="S")
        # stage-2 input: X4[8pi + q, r, u] = a3[128u + 8pi + q]
        X4 = spool.tile([128, rows, NB2], F16, name="X4")

        def stage2(h, row0, nrows):
            """Levels 4-7 for rows [row0, row0+nrows) (PE rebuild + matmuls)."""
            rs = slice(row0, row0 + nrows)
            px = p1pool.tile([128, nrows, NB2], F32, tag="ps1", name="px4")
            for pi in range(16):
                nc.tensor.matmul(px[:], WP[:, 128 * pi:128 * pi + 128],
                                 S[:, rs, pi, :], start=(pi == 0),
                                 stop=(pi == 15))
            if h % 2 == 0:
                nc.scalar.copy(X4[:, rs, :], px[:])
            else:
                nc.vector.tensor_copy(X4[:, rs, :], px[:])
            ps = p2pool.tile([128, nrows, NB2], F32, tag="ps2", name="ps2")
            nc.tensor.matmul(ps[:], M2, X4[:, rs, :], start=True, stop=False)
            nc.tensor.matmul(ps[:, :, 0:1], C2, X4[:, rs, NB2 - 1:NB2],
                             start=False, stop=False)
            nc.tensor.matmul(ps[:, :, 1:NB2], C2, X4[:, rs, 0:NB2 - 1],
                             start=False, stop=True)
            o = opool.tile([128, nrows, NB2], F16, tag="st2", name="o2")
            if h % 2 == 0:
                nc.vector.tensor_copy(o[:], ps[:])
            else:
                nc.scalar.copy(o[:], ps[:])
            nc.scalar.dma_start(st2r[:, rs, :], o[:])

        r0 = 0
        for ci, nr in enumerate(CHUNKS):
            rs = slice(r0, r0 + nr)
            x0 = xtiles[ci]
            tb = 512 // nr              # blocks per psum tile
            spt = NB // tb              # psum tiles this chunk (= nr)
            packs = [range(t0, min(t0 + 4, spt)) for t0 in range(0, spt, 4)]
            if ci == 0:
                # finish psum tile 0 from the head load alone, then keep the
                # PE busy (HAM warm) until the first full chunk lands
                packs = [range(0, 1), range(1, spt)]
            for pk, tts in enumerate(packs):
                if ci == 0 and pk == 1:
                    for _ in range(14):
                        nc.tensor.matmul(pw[:], warm[:, 0:128], warm[:],
                                         start=True, stop=True)
                pss = {}
                for t in tts:           # M-batch (one LDWEIGHTS)
                    b0 = t * tb
                    xs = xh if ci == 0 and t == 0 and tb == 128 else x0
                    ps = pss[t] = p1pool.tile([128, nr, tb], F32, tag="ps1",
                                              name="ps1")
                    nc.tensor.matmul(ps[:], M1, xs[:, :, 1 + b0:1 + b0 + tb],
                                     start=True, stop=False)
                for t in tts:           # C-batch
                    b0 = t * tb
                    xs = xh if ci == 0 and t == 0 and tb == 128 else x0
                    nc.tensor.matmul(pss[t][:], C1, xs[:, :, b0:b0 + tb],
                                     start=False, stop=True)
                for t in tts:           # evac f32->f16, pi-major S cols
                    u0 = (t * tb) // 16
                    dst = S[:, rs, :, u0:u0 + tb // 16]
                    src = pss[t][:].rearrange("p r (u pi) -> p r pi u", pi=16)
                    if t % 2 == 0:
                        nc.vector.tensor_copy(dst, src)
                    else:
                        nc.scalar.copy(dst, src)
            nc.sync.dma_start(st1r[:, rs, :], S[0:120, rs, :, :])
            if ci in QBOUND:
                qr0, qnr = QBOUND[ci]
                stage2(qr0 // 16, qr0, qnr)
            r0 += nr


_MODULE_CACHE = {}


def _get_module():
    if "nc" in _MODULE_CACHE:
        return _MODULE_CACHE["nc"]
    nc = bacc.Bacc("TRN2", target_bir_lowering=False, debug=False,
                   num_devices=N_CORES)
    xt = nc.dram_tensor("xt", [128, ROWS * (NB + 1)], F16,
                        kind="ExternalInput").ap()
    xhead = nc.dram_tensor("xhead", [128, 4 * 129], F16,
                           kind="ExternalInput").ap()
    wmat = nc.dram_tensor("wmat", [128, 2560], F16, kind="ExternalInput").ap()
    st1 = nc.dram_tensor("st1", [120, ROWS * NB], F16,
                         kind="ExternalOutput").ap()
    st2 = nc.dram_tensor("st2", [128, ROWS * NB2], F16,
                         kind="ExternalOutput").ap()
    with tile.TileContext(nc) as tc:
        _build_dwt(tc, xt, xhead, wmat, st1, st2)
    nc.compile()
    _MODULE_CACHE["nc"] = nc
    return nc


def run(x, scaling, **spmd_kwargs):
    """Full pipeline.  Returns (denoised, coeffs, BassKernelResults)."""
    x = np.ascontiguousarray(np.asarray(x, dtype=np.float32))
    scaling = np.asarray(scaling, dtype=np.float32)
    assert x.shape == (N_ROWS, N0), x.shape
    assert scaling.shape == (LEVELS, 8), scaling.shape

    nc = _get_module()
    wmat = _make_wmat(scaling)
    in_maps = []
    for c in range(N_CORES):
        xp = _pack_x_shard(x[c * ROWS:(c + 1) * ROWS])
        xhead = np.ascontiguousarray(
            xp.reshape(128, ROWS, NB + 1)[:, 0:4, 0:129].reshape(128, -1))
        in_maps.append({"xt": xp, "xhead": xhead, "wmat": wmat})

    res = run_bass_kernel_spmd(nc, in_maps, core_ids=list(range(N_CORES)),
                               **spmd_kwargs)

    coeffs = np.empty((N_ROWS, N0), dtype=np.float32)
    ds_full = []
    offs = np.cumsum([0] + [N0 >> (l + 1) for l in range(LEVELS)])
    for c in range(N_CORES):
        rsl = slice(c * ROWS, (c + 1) * ROWS)
        b1 = _unpack_stage(res.results[c]["st1"], ROWS, NB, True)
        b2 = _unpack_stage(res.results[c]["st2"], ROWS, NB2, False)
        for lvl in range(LEVELS):
            blk, (p0, cnt, _) = (b1, SEG[lvl]) if lvl < 4 else \
                (b2, SEG[lvl - 4])
            coeffs[rsl, offs[lvl]:offs[lvl + 1]] = (
                blk[:, :, p0:p0 + cnt].reshape(ROWS, -1))
        coeffs[rsl, offs[8]:] = b2[:, :, 120:128].reshape(ROWS, -1)

    for lvl in range(LEVELS):
        ds_full.append(coeffs[:, offs[lvl]:offs[lvl + 1]])
    a_full = coeffs[:, offs[8]:]

    if _is_orthonormal_qmf(scaling):
        # Orthonormal QMF bank + untouched coefficients => the inverse
        # transform is exactly the identity (reference pad is a no-op).
        denoised = x.copy()
    else:
        denoised = _dwt_backward_numpy(ds_full, a_full, scaling).astype(np.float32)

    return denoised, coeffs, res


def kernel(x, scaling):
    denoised, coeffs, _ = run(x, scaling)
    return denoised, coeffs


# revision 47
# speedup vs baseline: 1.1595x; 1.1518x over previous
"""Trainium2 Bass kernel for an 8-level circular DWT (forward + inverse).

The reference computes an 8-level periodized DWT (8-tap filters derived from
`scaling`) and returns (denoised, concat(coeffs)).  The inverse transform is
applied with no thresholding, so for orthonormal QMF filters (the DB4 bank
the reference ships) reconstruction is exactly the identity: denoised == x.
The kernel verifies that condition numerically and short-circuits the inverse
to a host-side copy; the forward transform runs on 8 NeuronCores,
data-parallel over rows (64 rows/core).

Device math: levels 0-3 are fused into ONE banded-matmul pass over x
("stage 1") using composite filters (up to 106 taps, stride 16 for the
level-3 outputs).  Output block c (128 outputs: 64 d0 | 32 d1 | 16 d2 |
8 d3 | 8 a3, at fixed partition segments) is

    psum[:, c] = M.T @ X_c + C.T @ X_{c-1}          (X_b = x[128b .. 128b+128))

so each input block is streamed exactly twice and both stationaries are
block-independent.  Matmuls are batched M,M,M,M / C,C,C,C so LDWEIGHTS
amortizes.  PSUM is evacuated f32->f16 by full-128-partition copies into S
with pi-major column order (block 16u + pi stored at col 32*pi + u), so the
a3 stripe (partitions 120:128) forms 64B-contiguous runs per pi.  One
SBUF->SBUF DMA per 16-row quarter then remaps the stripes into
X4[16q + pi, r, u] = a3[128u + 8pi + q] and stage 2 applies the identical
fused scheme for levels 4-7 on X4 (M2/C2 rows permuted to match the (q,pi)
partition order; 3 matmuls per quarter: M2, C2-wrap, C2).  Matmuls run in
float16 (full PE rate); PSUM accumulates fp32.
"""

import sys
from contextlib import ExitStack

for _p in ("/opt/trn_rl_repo", "/root/.axon_site/_ro/trn_rl_repo"):
    if _p not in sys.path:
        sys.path.append(_p)

import numpy as np

import concourse.bacc as bacc
import concourse.mybir as mybir
import concourse.tile as tile
from concourse.bass_utils import run_bass_kernel_spmd

F32 = mybir.dt.float32
F16 = mybir.dt.float16

N_ROWS = 512          # total rows
N0 = 65536            # row length (power of two: reference pad is a no-op)
LEVELS = 8
N_CORES = 8
ROWS = N_ROWS // N_CORES   # rows per core
NB = N0 // 128             # 128-blocks per row (512)
NB2 = NB // 16             # stage-2 blocks per row (32)
CHUNKS = (4, 4, 4, 4, 8, 8, 8, 8, 4, 4, 4, 4)  # stage-1 rows per chunk
# chunk idx -> stage-2 pieces (row0, nrows); emitted one chunk after the
# piece's rows are evacuated so its matmuls never stall the PE FIFO
QBOUND = {4: ((0, 16),), 6: ((16, 16),), 8: ((32, 16),), 10: ((48, 8),),
          11: ((56, 4), (60, 4))}

# output partition segments within a 128-output block (stage 1 and stage 2)
SEG = ((0, 64, 2), (64, 32, 4), (96, 16, 8), (112, 8, 16), (120, 8, 16))


# ----------------------------- host-side math -----------------------------

def _wavelet(s):
    g = s[::-1].copy()
    sign = np.where(np.arange(s.shape[-1]) % 2 == 1, -1.0, 1.0).astype(g.dtype)
    return g * sign


def _make_mc(filters):
    """Fused 4-level stationaries [M, C] (128x128 f64 each, [p_in, m] lhsT).

    filters: (4, 8) scaling rows for the 4 levels of this stage.  Output
    block families: d0 (stride 2, 64/block), d1 (4, 32), d2 (8, 16),
    d3 (16, 8), a3 (16, 8) at partition bases 0/64/96/112/120.
    out[j] = sum_t g[t] x[s*j - t]; in-block index i = s*q - t for block
    slot q; i < 0 reads the previous block via C.
    """
    fs = [np.asarray(f, dtype=np.float64) for f in filters]
    ws = [_wavelet(f) for f in fs]
    # P[l] = taps of a_l w.r.t. stage input: P_l[2^l m + t] += s_l[m] P_{l-1}[t]
    P = [np.array([1.0])]
    for lvl in range(3):
        q = np.zeros((1 << lvl) * 7 + len(P[-1]), dtype=np.float64)
        for m in range(8):
            q[(1 << lvl) * m:(1 << lvl) * m + len(P[-1])] += fs[lvl][m] * P[-1]
        P.append(q)
    taps = []
    for lvl in range(4):  # d-taps per level
        g = np.zeros((1 << lvl) * 7 + len(P[lvl]), dtype=np.float64)
        for m in range(8):
            g[(1 << lvl) * m:(1 << lvl) * m + len(P[lvl])] += ws[lvl][m] * P[lvl]
        taps.append(g)
    ga = np.zeros(8 * 7 + len(P[3]), dtype=np.float64)  # a3-taps
    for m in range(8):
        ga[8 * m:8 * m + len(P[3])] += fs[3][m] * P[3]
    taps.append(ga)

    M = np.zeros((128, 128), dtype=np.float64)
    C = np.zeros((128, 128), dtype=np.float64)
    for (pbase, cnt, s), g in zip(SEG, taps):
        for q in range(cnt):
            for t in range(len(g)):
                i = s * q - t
                if i >= 0:
                    M[i, pbase + q] += g[t]
                else:
                    C[i + 128, pbase + q] += g[t]
    return M, C


def _make_wmat(scaling):
    s = np.asarray(scaling, dtype=np.float64)
    M1, C1 = _make_mc(s[0:4])
    M2, C2 = _make_mc(s[4:8])
    # rebuild permutation matmuls: X4[8pi+q] <- S partition 120+q, pi-group
    WP = np.zeros((128, 16 * 128))
    for pi in range(16):
        for q in range(8):
            WP[120 + q, 128 * pi + 8 * pi + q] = 1.0
    return np.concatenate([M1, C1, M2, C2, WP], axis=1).astype(np.float16)


def _pack_x_shard(x_rows):
    rows, n = x_rows.shape
    nb = n // 128
    blocks = x_rows.astype(np.float16).reshape(rows, nb, 128).transpose(2, 0, 1)
    xt = np.empty((128, rows, nb + 1), dtype=np.float16)
    xt[:, :, 1:] = blocks
    xt[:, :, 0] = blocks[:, :, nb - 1]           # circular halo column
    return np.ascontiguousarray(xt.reshape(128, rows * (nb + 1)))


def _unpack_stage(arr, rows, nblk, pi_major):
    """[P, rows*nblk-cols] device layout -> [rows, nblk, P] block-major."""
    p = arr.shape[0]
    if pi_major:  # device cols (pi, u); block b = 16u + pi
        a = arr.reshape(p, rows, 16, nblk // 16).transpose(1, 3, 2, 0)
    else:
        a = arr.reshape(p, rows, nblk).transpose(1, 2, 0)
    return np.ascontiguousarray(a.reshape(rows, nblk, p))


def _is_orthonormal_qmf(scaling):
    s = np.asarray(scaling, dtype=np.float64)
    if s.shape != (LEVELS, 8):
        return False
    for lvl in range(LEVELS):
        f = s[lvl]
        for m in range(4):
            v = np.dot(f[: 8 - 2 * m], f[2 * m:])
            if abs(v - (1.0 if m == 0 else 0.0)) > 1e-4:
                return False
    return True


def _dwt_backward_numpy(ds, a, scaling):
    """Fallback inverse transform (float64 FFT) for non-orthonormal filters."""
    a = np.asarray(a, dtype=np.float64)
    for lvl in reversed(range(LEVELS)):
        s = np.asarray(scaling[lvl], dtype=np.float64)
        w = _wavelet(s)
        d = np.asarray(ds[lvl], dtype=np.float64)
        n = d.shape[-1] * 2
        fd = np.zeros((d.shape[0], n))
        fd[:, ::2] = d
        fa = np.zeros((a.shape[0], n))
        fa[:, ::2] = a
        a = (np.fft.irfft(np.fft.rfft(fd, axis=-1)
                          * np.conj(np.fft.rfft(w, n=n)), n=n, axis=-1)
             + np.fft.irfft(np.fft.rfft(fa, axis=-1)
                            * np.conj(np.fft.rfft(s, n=n)), n=n, axis=-1))
    return a


# ----------------------------- device kernel ------------------------------

def _build_dwt(tc, xt, xhead, wmat, st1, st2, rows=ROWS):
    nc = tc.nc
    xt3 = xt.rearrange("p (r b) -> p r b", b=NB + 1)
    st1r = st1.rearrange("p (r c) -> p r c", c=NB)
    st2r = st2.rearrange("p (r c) -> p r c", c=NB2)

    with ExitStack() as ctx:
        wpool = ctx.enter_context(tc.tile_pool(name="wpool", bufs=1))
        xpool = ctx.enter_context(tc.tile_pool(name="xpool",
                                               bufs=len(CHUNKS)))
        spool = ctx.enter_context(tc.tile_pool(name="spool", bufs=1))
        opool = ctx.enter_context(tc.tile_pool(name="opool", bufs=2))
        p1pool = ctx.enter_context(tc.tile_pool(name="p1pool", bufs=6,
                                                space="PSUM"))
        p2pool = ctx.enter_context(tc.tile_pool(name="p2pool", bufs=2,
                                                space="PSUM"))

        # weights on the (otherwise idle at start) scalar queue
        W = wpool.tile([128, 512], F16, name="Wsb")
        nc.scalar.dma_start(W[:], wmat[:, 0:512])
        M1, C1 = W[:, 0:128], W[:, 128:256]
        M2, C2 = W[:, 256:384], W[:, 384:512]
        WP = wpool.tile([128, 2048], F16, name="WPsb")
        nc.scalar.dma_start(WP[:], wmat[:, 512:2560])

        # tiny head load: lets the first psum tile start ~3us earlier
        xh = wpool.tile([128, 4, 129], F16, name="xh")
        nc.sync.dma_start(xh[:], xhead[:, :].rearrange("p (r b) -> p r b",
                                                       b=129))

        # all input loads issued up front, alternating queues so the early
        # chunks stream in parallel instead of serializing their latency
        xtiles = []
        r0 = 0
        for ci, nr in enumerate(CHUNKS):
            x0 = xpool.tile([128, nr, NB + 1], F16, tag="x0",
                            name=f"x{ci}")
            nc.sync.dma_start(x0[:], xt3[:, r0:r0 + nr, :])
            xtiles.append(x0)
            r0 += nr

        # PE warm-up: keep HAM unthrottled until the first input lands.
        warm = wpool.tile([128, 256], F16, name="warm")
        nc.gpsimd.memset(warm[:], 0)
        pw = p1pool.tile([128, 256], F32, tag="ps1", name="pw")
        for _ in range(12):
            nc.tensor.matmul(pw[:], warm[:, 0:128], warm[:], start=True,
                             stop=True)

        # stage-1 output: S[p, r, pi, u] holds block 16u + pi of row r
        S = spool.tile([128, rows, 16, NB // 16], F16, name="S")
        # stage-2 input: X4[8pi + q, r, u] = a3[128u + 8pi + q]
        X4 = spool.tile([128, rows, NB2], F16, name="X4")

        def stage2(h, row0, nrows):
            """Levels 4-7 for rows [row0, row0+nrows) (PE rebuild + matmuls)."""
            rs = slice(row0, row0 + nrows)
            px = p1pool.tile([128, nrows, NB2], F32, tag="ps1", name="px4")
            for pi in range(16):
                nc.tensor.matmul(px[:], WP[:, 128 * pi:128 * pi + 128],
                                 S[:, rs, pi, :], start=(pi == 0),
                                 stop=(pi == 15))
            if h % 2 == 0:
                nc.scalar.copy(X4[:, rs, :], px[:])
            else:
                nc.vector.tensor_copy(X4[:, rs, :], px[:])
            ps = p2pool.tile([128, nrows, NB2], F32, tag="ps2", name="ps2")
            nc.tensor.matmul(ps[:], M2, X4[:, rs, :], start=True, stop=False)
            nc.tensor.matmul(ps[:, :, 0:1], C2, X4[:, rs, NB2 - 1:NB2],
                             start=False, stop=False)
            nc.tensor.matmul(ps[:, :, 1:NB2], C2, X4[:, rs, 0:NB2 - 1],
                             start=False, stop=True)
            o = opool.tile([128, nrows, NB2], F16, tag="st2", name="o2")
            if h % 2 == 0:
                nc.vector.tensor_copy(o[:], ps[:])
            else:
                nc.scalar.copy(o[:], ps[:])
            nc.scalar.dma_start(st2r[:, rs, :], o[:])

        r0 = 0
        for ci, nr in enumerate(CHUNKS):
            rs = slice(r0, r0 + nr)
            x0 = xtiles[ci]
            tb = 512 // nr              # blocks per psum tile
            spt = NB // tb              # psum tiles this chunk (= nr)
            packs = [range(t0, min(t0 + 4, spt)) for t0 in range(0, spt, 4)]
            if ci == 0:
                # finish psum tile 0 from the head load alone, then keep the
                # PE busy (HAM warm) until the first full chunk lands
                packs = [range(0, 1), range(1, spt)]
            for pk, tts in enumerate(packs):
                if ci == 0 and pk == 1:
                    for _ in range(14):
                        nc.tensor.matmul(pw[:], warm[:, 0:128], warm[:],
                                         start=True, stop=True)
                pss = {}
                for t in tts:           # M-batch (one LDWEIGHTS)
                    b0 = t * tb
                    xs = xh if ci == 0 and t == 0 and tb == 128 else x0
                    ps = pss[t] = p1pool.tile([128, nr, tb], F32, tag="ps1",
                                              name="ps1")
                    nc.tensor.matmul(ps[:], M1, xs[:, :, 1 + b0:1 + b0 + tb],
                                     start=True, stop=False)
                for t in tts:           # C-batch
                    b0 = t * tb
                    xs = xh if ci == 0 and t == 0 and tb == 128 else x0
                    nc.tensor.matmul(pss[t][:], C1, xs[:, :, b0:b0 + tb],
                                     start=False, stop=True)
                for t in tts:           # evac f32->f16, pi-major S cols
                    u0 = (t * tb) // 16
                    dst = S[:, rs, :, u0:u0 + tb // 16]
                    src = pss[t][:].rearrange("p r (u pi) -> p r pi u", pi=16)
                    if t % 2 == 0:
                        nc.vector.tensor_copy(dst, src)
                    else:
                        nc.scalar.copy(dst, src)
            nc.sync.dma_start(st1r[:, rs, :], S[0:120, rs, :, :])
            for qr0, qnr in QBOUND.get(ci, ()):
                stage2(qr0 // 16, qr0, qnr)
            r0 += nr


_MODULE_CACHE = {}


def _get_module():
    if "nc" in _MODULE_CACHE:
        return _MODULE_CACHE["nc"]
    nc = bacc.Bacc("TRN2", target_bir_lowering=False, debug=False,
                   num_devices=N_CORES)
    xt = nc.dram_tensor("xt", [128, ROWS * (NB + 1)], F16,
                        kind="ExternalInput").ap()
    xhead = nc.dram_tensor("xhead", [128, 4 * 129], F16,
                           kind="ExternalInput").ap()
    wmat = nc.dram_tensor("wmat", [128, 2560], F16, kind="ExternalInput").ap()
    st1 = nc.dram_tensor("st1", [120, ROWS * NB], F16,
                         kind="ExternalOutput").ap()
    st2 = nc.dram_tensor("st2", [128, ROWS * NB2], F16,
                         kind="ExternalOutput").ap()
    with tile.TileContext(nc) as tc:
        _build_dwt(tc, xt, xhead, wmat, st1, st2)
    nc.compile()
    _MODULE_CACHE["nc"] = nc
    return nc


def run(x, scaling, **spmd_kwargs):
    """Full pipeline.  Returns (denoised, coeffs, BassKernelResults)."""
    x = np.ascontiguousarray(np.asarray(x, dtype=np.float32))
    scaling = np.asarray(scaling, dtype=np.float32)
    assert x.shape == (N_ROWS, N0), x.shape
    assert scaling.shape == (LEVELS, 8), scaling.shape

    nc = _get_module()
    wmat = _make_wmat(scaling)
    in_maps = []
    for c in range(N_CORES):
        xp = _pack_x_shard(x[c * ROWS:(c + 1) * ROWS])
        xhead = np.ascontiguousarray(
            xp.reshape(128, ROWS, NB + 1)[:, 0:4, 0:129].reshape(128, -1))
        in_maps.append({"xt": xp, "xhead": xhead, "wmat": wmat})

    res = run_bass_kernel_spmd(nc, in_maps, core_ids=list(range(N_CORES)),
                               **spmd_kwargs)

    coeffs = np.empty((N_ROWS, N0), dtype=np.float32)
    ds_full = []
    offs = np.cumsum([0] + [N0 >> (l + 1) for l in range(LEVELS)])
    for c in range(N_CORES):
        rsl = slice(c * ROWS, (c + 1) * ROWS)
        b1 = _unpack_stage(res.results[c]["st1"], ROWS, NB, True)
        b2 = _unpack_stage(res.results[c]["st2"], ROWS, NB2, False)
        for lvl in range(LEVELS):
            blk, (p0, cnt, _) = (b1, SEG[lvl]) if lvl < 4 else \
                (b2, SEG[lvl - 4])
            coeffs[rsl, offs[lvl]:offs[lvl + 1]] = (
                blk[:, :, p0:p0 + cnt].reshape(ROWS, -1))
        coeffs[rsl, offs[8]:] = b2[:, :, 120:128].reshape(ROWS, -1)

    for lvl in range(LEVELS):
        ds_full.append(coeffs[:, offs[lvl]:offs[lvl + 1]])
    a_full = coeffs[:, offs[8]:]

    if _is_orthonormal_qmf(scaling):
        # Orthonormal QMF bank + untouched coefficients => the inverse
        # transform is exactly the identity (reference pad is a no-op).
        denoised = x.copy()
    else:
        denoised = _dwt_backward_numpy(ds_full, a_full, scaling).astype(np.float32)

    return denoised, coeffs, res


def kernel(x, scaling):
    denoised, coeffs, _ = run(x, scaling)
    return denoised, coeffs
